# revision 15
# baseline (speedup 1.0000x reference)
"""Self-contained TRN2 Bass kernel for nn_FLoRALayer (B=8, S=2048, D=1024, R=8).

kernel(**inputs) takes FULL unsharded inputs:
    x         [8, 2048, 1024] f32
    adapter_b [8, 1024, 8]    f32
    adapter_a [8, 8, 1024]    f32
    W0        [1024, 1024]    f32
returns the FULL [8, 2048, 1024] f32 output of:
    BxW0 = einsum('bsd,bdr,do->bsro', x.astype(fp16), adapter_b, W0)
    out  = relu(mean(swapaxes(adapter_a,1,2)[:,None]*BxW0.reshape(b,s,d,r), -1))

Math refactor (verified exactly): with o = kk*128 + g*16 + mp,
    W_eff[dd, o] = adapter_b[dd, kk] * sum_rp adapter_a[rp, o] * W0[dd, (o%128)*8 + rp]
    out[b] = relu((x_fp16[b] @ W_eff[b]) / 8)
one [2048,1024] @ [1024,1024] fp16 matmul per batch; batch b on NeuronCore b.

v4 schedule, from trace analysis (PE floor ~59us at 216ns per 512-row fp16
matmul; HBM floor ~56us; one DMA ring sustains only ~213GB/s, two ~358GB/s):
  - weff kept in PSUM-native column order j = g*128+kk*16+mp; host unpermutes
    the output columns (pure data movement), making the BTT contiguous.
  - adapter uploads are compact and merged into ONE [128,144] tensor
    (aspc|mask|bcp); the block-diagonal asp matmul operand is built on-device
    with one DVE multiply (aspc broadcast over mp) x (mask broadcast g,kk).
  - reads ride TWO rings so the weff chain (which gates every non-warm main)
    completes at full aggregate bandwidth ~23us:
      sync ring:   consts, w0_0, x0, w0_2, w0_4, w0_6, x2, x4, ..., x14
      gpsimd ring: x1, w0_1, w0_3, w0_5, w0_7, x3, x5, ..., x15
    Output stores ride the scalar (even halves) + gpsimd (odd halves) rings,
    which are idle by the time evacs start.
  - w0 f32->fp16 casts on ACT (keeps DVE queue free for BTTs); x casts on
    DVE, placed so no cast ever heads the queue before a chain-critical BTT.
  - PSUM: 7 "po" half-banks (warm s0/s1 both halves + steady cycling) +
    1 "pc" bank; each C chunk runs as two 4-matmul halves with warm mains
    woven between.
"""

from contextlib import ExitStack

import numpy as np

S, D, R = 2048, 1024, 8
NT = D // 128
NS = S // 128
N_CORES = 8

_compiled = None


def _build_kernel():
    import concourse.bass as bass
    import concourse.tile as tile
    from concourse import bacc, mybir

    F32 = mybir.dt.float32
    F16 = mybir.dt.float16

    nc = bacc.Bacc(
        "TRN2", target_bir_lowering=False, debug=False, num_devices=N_CORES
    )

    x_d = nc.dram_tensor("xtp", [NS, 128, D], F32, kind="ExternalInput").ap()
    w0_d = nc.dram_tensor("w0tp", [NT, 128, D], F32, kind="ExternalInput").ap()
    cst_d = nc.dram_tensor("cpack", [128, 144], F32, kind="ExternalInput").ap()
    out_d = nc.dram_tensor("out", [S, D], F32, kind="ExternalOutput").ap()

    with tile.TileContext(nc) as tc, ExitStack() as ctx:
        pool = lambda name, bufs, **kw: ctx.enter_context(
            tc.tile_pool(name=name, bufs=bufs, **kw)
        )
        const_p = pool("const", 1)
        big_p = pool("big", 1)
        outst_p = pool("outst", 5)
        pmm_p = pool("pmm", 1, space="PSUM")

        xall = big_p.tile([128, NS * D], F32, tag="xall")
        xth = big_p.tile([128, NS * D], F16, tag="xth")
        w0all = big_p.tile([128, NT * D], F32, tag="w0all")
        w0h = big_p.tile([128, NT * D], F16, tag="w0h")
        weff = big_p.tile([128, NT * D], F16, tag="weff")
        cpack = const_p.tile([128, 144], F32, tag="cpack")
        asph = const_p.tile([128, NT * 128], F16, tag="asph")
        aspc = cpack[:, 0:64]
        mmask = cpack[:, 64:80]
        bcp = cpack[:, 80:144]

        # ---- reads on two rings (aggregate ~358GB/s needs both busy) ----
        def w0_dma(ring, t):
            ring.dma_start(w0all[:, t * D : (t + 1) * D], w0_d[t])

        def x_dma(ring, s):
            ring.dma_start(xall[:, s * D : (s + 1) * D], x_d[s])

        nc.sync.dma_start(cpack[:], cst_d[:])
        w0_dma(nc.sync, 0)
        x_dma(nc.sync, 0)
        x_dma(nc.gpsimd, 1)
        for t in (1, 3, 5, 7):
            w0_dma(nc.gpsimd, t)
        for t in (2, 4, 6):
            w0_dma(nc.sync, t)
        for s in range(2, NS, 2):
            x_dma(nc.sync, s)
        for s in range(3, NS, 2):
            x_dma(nc.gpsimd, s)

        # ---- asp scatter: asph[p, g*128+kk*16+mp] = aspc[p,g*8+kk]*mask[p,mp]
        nc.vector.tensor_tensor(
            out=asph[:].rearrange("p (g kk mp) -> p g kk mp", g=8, kk=8),
            in0=aspc.rearrange("p (g kk) -> p g kk", g=8)[:, :, :, None]
            .broadcast_to([128, 8, 8, 16]),
            in1=mmask[:, None, None, :].broadcast_to([128, 8, 8, 16]),
            op=mybir.AluOpType.mult,
        )

        xc_done = set()

        def x_cast(s):
            if 0 <= s < NS and s not in xc_done:
                xc_done.add(s)
                nc.vector.tensor_copy(
                    xth[:, s * D : (s + 1) * D], xall[:, s * D : (s + 1) * D]
                )

        def x_op(s):
            return xth[:, s * D : (s + 1) * D]

        def weff_op(c, h):
            return weff[:, c * D + h * 512 : c * D + (h + 1) * 512]

        # warm units: (s, h) psum halves in flight during the weff chain
        WARMH = [(1, 0), (1, 1), (0, 0), (0, 1)]
        po_warm = {
            (s, h): pmm_p.tile(
                [128, 512], F32, tag="po", bufs=7, name=f"po{s}_{h}"
            )
            for (s, h) in WARMH
        }

        def warm_unit(s, h, c):
            nc.tensor.matmul(
                po_warm[(s, h)][:],
                lhsT=x_op(s)[:, c * 128 : (c + 1) * 128],
                rhs=weff_op(c, h),
                start=(c == 0),
                stop=(c == NT - 1),
            )

        # ---- weff chain ----
        for t in range(NT):
            nc.scalar.activation(
                w0h[:, t * D : (t + 1) * D],
                w0all[:, t * D : (t + 1) * D],
                mybir.ActivationFunctionType.Copy,
                scale=1.0,
            )
            for half in range(2):
                pc = pmm_p.tile(
                    [128, 512], F32, tag="pc", bufs=1, name=f"pc{t}_{half}"
                )
                for gg in range(4):
                    g = half * 4 + gg
                    nc.tensor.matmul(
                        pc[:, gg * 128 : (gg + 1) * 128],
                        lhsT=w0h[:, t * D + g * 128 : t * D + (g + 1) * 128],
                        rhs=asph[:, g * 128 : (g + 1) * 128],
                        start=True,
                        stop=True,
                    )
                wv = weff[
                    :, t * D + half * 512 : t * D + (half + 1) * 512
                ].rearrange("p (g kk mp) -> p g kk mp", g=4, kk=8)
                pv = pc[:].rearrange("p (g kk mp) -> p g kk mp", g=4, kk=8)
                bv = bcp[:, t * 8 : (t + 1) * 8][:, None, :, None].broadcast_to(
                    [128, 4, 8, 16]
                )
                nc.vector.tensor_tensor(
                    out=wv, in0=pv, in1=bv, op=mybir.AluOpType.mult
                )
                # warm mains for the previous chunk, two per C-half
                if t > 0:
                    for s, h in WARMH[2 * half : 2 * half + 2]:
                        warm_unit(s, h, t - 1)
            # x casts for s0/s1 placed after the first chunk's BTTs so they
            # never block chain-critical DVE work
            if t == 0:
                x_cast(0)
                x_cast(1)
        for s, h in WARMH:
            warm_unit(s, h, NT - 1)

        def evac(po, s):
            outst = outst_p.tile([128, D], F32, tag="outst", name=f"o{s}")
            nc.scalar.activation(
                outst[:, 0:512],
                po[0][:],
                mybir.ActivationFunctionType.Relu,
                scale=0.125,
            )
            nc.scalar.dma_start(
                out_d[s * 128 : (s + 1) * 128, 0:512], outst[:, 0:512]
            )
            nc.vector.tensor_scalar(
                out=outst[:, 512:1024],
                in0=po[1][:],
                scalar1=0.125,
                scalar2=0.0,
                op0=mybir.AluOpType.mult,
                op1=mybir.AluOpType.max,
            )
            nc.gpsimd.dma_start(
                out_d[s * 128 : (s + 1) * 128, 512:1024], outst[:, 512:1024]
            )

        # warm tiles: evac as soon as the chain completes
        for s in (0, 1):
            evac([po_warm[(s, 0)], po_warm[(s, 1)]], s)

        # ---- steady mains + evac/store ----
        for s in range(2, NS):
            x_cast(s)
            x_cast(s + 1)
            x_cast(s + 2)
            po = [
                pmm_p.tile([128, 512], F32, tag="po", bufs=7, name=f"po{s}_{h}")
                for h in range(2)
            ]
            for c in range(NT):
                for h in range(2):
                    nc.tensor.matmul(
                        po[h][:],
                        lhsT=x_op(s)[:, c * 128 : (c + 1) * 128],
                        rhs=weff_op(c, h),
                        start=(c == 0),
                        stop=(c == NT - 1),
                    )
            evac(po, s)

    nc.compile()
    return nc


# output column unpermute: stored col j=(g,kk,mp) -> true col o=kk*128+g*16+mp
_j = np.arange(D)
_PERM = ((_j >> 4) & 7) * 128 + (_j >> 7) * 16 + (_j & 15)
_INV = np.argsort(_PERM)  # out_full[..., o] = stored[..., _INV[o]]


def _pack_inputs(x_b, adapter_b_b, adapter_a_b, W0):
    """Pure data placement (permutation / replication / zero-padding)."""
    xtp = np.ascontiguousarray(
        x_b.reshape(NS, 128, NT, 128).transpose(0, 3, 2, 1).reshape(NS, 128, D),
        np.float32,
    )
    w0tp = np.ascontiguousarray(
        W0.reshape(NT, 128, NT, 128).transpose(0, 3, 2, 1).reshape(NT, 128, D),
        np.float32,
    )
    # aspc[mp*8+rp, g*8+kk] = a[rp, kk*128+g*16+mp]
    mp_i, rp_i, g_i, kk_i = np.meshgrid(
        np.arange(16), np.arange(8), np.arange(8), np.arange(8), indexing="ij"
    )
    aspc = adapter_a_b[rp_i, kk_i * 128 + g_i * 16 + mp_i].reshape(128, 64)
    mmask = (
        np.arange(16)[None, :] == (np.arange(128)[:, None] // 8)
    ).astype(np.float32)
    # bcp[dp, t*8+kk] = b[t*128+dp, kk]
    bcp = adapter_b_b.reshape(NT, 128, R).transpose(1, 0, 2).reshape(128, NT * R)
    cpack = np.ascontiguousarray(
        np.concatenate([aspc, mmask, bcp], axis=1), np.float32
    )
    return {"xtp": xtp, "w0tp": w0tp, "cpack": cpack}


def kernel(x, adapter_b, adapter_a, W0):
    global _compiled
    x = np.asarray(x, np.float32)
    adapter_b = np.asarray(adapter_b, np.float32)
    adapter_a = np.asarray(adapter_a, np.float32)
    W0 = np.asarray(W0, np.float32)
    B = x.shape[0]
    assert B == N_CORES and x.shape == (B, S, D)

    if _compiled is None:
        _compiled = _build_kernel()

    from concourse.bass_utils import run_bass_kernel_spmd

    in_maps = [
        _pack_inputs(x[b], adapter_b[b], adapter_a[b], W0) for b in range(B)
    ]
    res = run_bass_kernel_spmd(_compiled, in_maps, list(range(N_CORES)))
    out = np.stack([res.results[b]["out"] for b in range(B)]).astype(np.float32)
    return out[:, :, _INV]


# revision 17
# speedup vs baseline: 1.0712x; 1.0712x over previous
"""Self-contained TRN2 Bass kernel for nn_FLoRALayer (B=8, S=2048, D=1024, R=8).

kernel(**inputs) takes FULL unsharded inputs:
    x         [8, 2048, 1024] f32
    adapter_b [8, 1024, 8]    f32
    adapter_a [8, 8, 1024]    f32
    W0        [1024, 1024]    f32
returns the FULL [8, 2048, 1024] f32 output of:
    BxW0 = einsum('bsd,bdr,do->bsro', x.astype(fp16), adapter_b, W0)
    out  = relu(mean(swapaxes(adapter_a,1,2)[:,None]*BxW0.reshape(b,s,d,r), -1))

Math refactor (verified exactly): with o = kk*128 + g*16 + mp,
    W_eff[dd, o] = adapter_b[dd, kk] * sum_rp adapter_a[rp, o] * W0[dd, (o%128)*8 + rp]
    out[b] = relu((x_fp16[b] @ W_eff[b]) / 8)
one [2048,1024] @ [1024,1024] fp16 matmul per batch; batch b on NeuronCore b.

v5 schedule, from trace analysis (PE floor ~59us at 216ns per 512-row fp16
matmul; HBM floor ~56us; one DMA ring at 4KB elems sustains only ~213GB/s):
  - x and W0 are host-packed COLUMN-CONCATENATED in SBUF layout
    ([128, 16*1024] / [128, 8*1024]) so any multi-tile window is contiguous
    per partition -> large-element DMAs at full bandwidth on a single ring,
    with exact priority control:
      cpack | w0_0 | x0 | x1 | w0_1,w0_2 | x2,x3 | w0_3,w0_4 | w0_5,w0_6 |
      w0_7 | x4,x5 | ... | x14,x15
  - weff kept in PSUM-native column order j = g*128+kk*16+mp; host unpermutes
    the output columns (pure data movement), making the BTT contiguous.
  - adapter uploads compact, merged into ONE [128,144] tensor (aspc|mask|bcp);
    the block-diagonal asp operand is built on-device with one DVE multiply.
  - warm set = 7 PSUM half-banks (s0,s1 both halves; s2 both; s3 half0) with
    a catch-up scheduler: later halves join as their x tile lands and replay
    missed chunks, keeping the PE saturated through the whole weff chain.
  - w0 f32->fp16 casts on ACT; BTTs + x casts on DVE ordered so no cast ever
    blocks a chain-critical BTT; evac relu halves on ACT+DVE; stores on the
    scalar (even) and gpsimd (odd) rings which are idle by evac time.
"""

from contextlib import ExitStack

import numpy as np

S, D, R = 2048, 1024, 8
NT = D // 128
NS = S // 128
N_CORES = 8

_compiled = None


def _build_kernel():
    import concourse.bass as bass
    import concourse.tile as tile
    from concourse import bacc, mybir

    F32 = mybir.dt.float32
    F16 = mybir.dt.float16

    nc = bacc.Bacc(
        "TRN2", target_bir_lowering=False, debug=False, num_devices=N_CORES
    )

    x_d = nc.dram_tensor("xcat", [128, NS * D], F32, kind="ExternalInput").ap()
    w0_d = nc.dram_tensor("w0cat", [128, NT * D], F32, kind="ExternalInput").ap()
    cst_d = nc.dram_tensor("cpack", [128, 144], F32, kind="ExternalInput").ap()
    out_d = nc.dram_tensor("out", [S, D], F32, kind="ExternalOutput").ap()

    with tile.TileContext(nc) as tc, ExitStack() as ctx:
        pool = lambda name, bufs, **kw: ctx.enter_context(
            tc.tile_pool(name=name, bufs=bufs, **kw)
        )
        const_p = pool("const", 1)
        big_p = pool("big", 1)
        outst_p = pool("outst", 5)
        pmm_p = pool("pmm", 1, space="PSUM")

        xall = big_p.tile([128, NS * D], F32, tag="xall")
        xth = big_p.tile([128, NS * D], F16, tag="xth")
        w0all = big_p.tile([128, NT * D], F32, tag="w0all")
        w0h = big_p.tile([128, NT * D], F16, tag="w0h")
        weff = big_p.tile([128, NT * D], F16, tag="weff")
        cpack = const_p.tile([128, 144], F32, tag="cpack")
        asph = const_p.tile([128, NT * 128], F16, tag="asph")
        aspc = cpack[:, 0:64]
        mmask = cpack[:, 64:80]
        bcp = cpack[:, 80:144]

        # ---- all reads on the sync ring, large-elem, priority order ----
        def w0_dma(t0, t1):
            nc.sync.dma_start(
                w0all[:, t0 * D : t1 * D], w0_d[:, t0 * D : t1 * D]
            )

        def x_dma(s0, s1):
            nc.sync.dma_start(
                xall[:, s0 * D : s1 * D], x_d[:, s0 * D : s1 * D]
            )

        nc.sync.dma_start(cpack[:], cst_d[:])
        w0_dma(0, 1)
        x_dma(0, 1)
        x_dma(1, 2)
        w0_dma(1, 3)
        x_dma(2, 4)
        w0_dma(3, 5)
        w0_dma(5, 7)
        w0_dma(7, 8)
        for s in range(4, NS, 2):
            x_dma(s, s + 2)

        # ---- asp scatter: asph[p, g*128+kk*16+mp] = aspc[p,g*8+kk]*mask[p,mp]
        nc.vector.tensor_tensor(
            out=asph[:].rearrange("p (g kk mp) -> p g kk mp", g=8, kk=8),
            in0=aspc.rearrange("p (g kk) -> p g kk", g=8)[:, :, :, None]
            .broadcast_to([128, 8, 8, 16]),
            in1=mmask[:, None, None, :].broadcast_to([128, 8, 8, 16]),
            op=mybir.AluOpType.mult,
        )

        xc_done = set()

        def x_cast(s):
            if 0 <= s < NS and s not in xc_done:
                xc_done.add(s)
                nc.vector.tensor_copy(
                    xth[:, s * D : (s + 1) * D], xall[:, s * D : (s + 1) * D]
                )

        def x_op(s):
            return xth[:, s * D : (s + 1) * D]

        def weff_op(c, h):
            return weff[:, c * D + h * 512 : c * D + (h + 1) * 512]

        # warm halves with join step + catch-up: (s, h) -> join step J;
        # at chain step t (1..8) a half emits its uncovered chunks up to t-1,
        # at most 3 per step. All 7 halves cover all 8 chunks by step 8.
        WARMH = [(0, 0), (0, 1), (1, 0), (1, 1), (2, 0), (2, 1), (3, 0)]
        JOIN = {(0, 0): 1, (0, 1): 1, (1, 0): 1, (1, 1): 1,
                (2, 0): 3, (2, 1): 3, (3, 0): 4}
        next_c = {k: 0 for k in WARMH}
        po_warm = {
            (s, h): pmm_p.tile(
                [128, 512], F32, tag="po", bufs=7, name=f"po{s}_{h}"
            )
            for (s, h) in WARMH
        }

        def warm_unit(s, h, c):
            nc.tensor.matmul(
                po_warm[(s, h)][:],
                lhsT=x_op(s)[:, c * 128 : (c + 1) * 128],
                rhs=weff_op(c, h),
                start=(c == 0),
                stop=(c == NT - 1),
            )

        def warm_step(t, part):
            # part 0 -> halves WARMH[0,2,4,6], part 1 -> WARMH[1,3,5]
            for s, h in WARMH[part::2]:
                if JOIN[(s, h)] > t:
                    continue
                hi = t if t <= NT else NT
                emitted = 0
                while next_c[(s, h)] < hi and emitted < 3:
                    warm_unit(s, h, next_c[(s, h)])
                    next_c[(s, h)] += 1
                    emitted += 1

        # ---- weff chain ----
        for t in range(NT):
            nc.scalar.activation(
                w0h[:, t * D : (t + 1) * D],
                w0all[:, t * D : (t + 1) * D],
                mybir.ActivationFunctionType.Copy,
                scale=1.0,
            )
            for half in range(2):
                pc = pmm_p.tile(
                    [128, 512], F32, tag="pc", bufs=1, name=f"pc{t}_{half}"
                )
                for gg in range(4):
                    g = half * 4 + gg
                    nc.tensor.matmul(
                        pc[:, gg * 128 : (gg + 1) * 128],
                        lhsT=w0h[:, t * D + g * 128 : t * D + (g + 1) * 128],
                        rhs=asph[:, g * 128 : (g + 1) * 128],
                        start=True,
                        stop=True,
                    )
                wv = weff[
                    :, t * D + half * 512 : t * D + (half + 1) * 512
                ].rearrange("p (g kk mp) -> p g kk mp", g=4, kk=8)
                pv = pc[:].rearrange("p (g kk mp) -> p g kk mp", g=4, kk=8)
                bv = bcp[:, t * 8 : (t + 1) * 8][:, None, :, None].broadcast_to(
                    [128, 4, 8, 16]
                )
                nc.vector.tensor_tensor(
                    out=wv, in0=pv, in1=bv, op=mybir.AluOpType.mult
                )
                if t > 0:
                    warm_step(t, half)
            if t == 0:
                x_cast(0)
                x_cast(1)
            elif t == 1:
                x_cast(2)
            elif t == 2:
                x_cast(3)
        warm_step(NT + 2, 0)
        warm_step(NT + 2, 1)
        for k in WARMH:
            assert next_c[k] == NT, (k, next_c[k])

        def evac(po, s):
            outst = outst_p.tile([128, D], F32, tag="outst", name=f"o{s}")
            nc.scalar.activation(
                outst[:, 0:512],
                po[0][:],
                mybir.ActivationFunctionType.Relu,
                scale=0.125,
            )
            nc.scalar.dma_start(
                out_d[s * 128 : (s + 1) * 128, 0:512], outst[:, 0:512]
            )
            nc.vector.tensor_scalar(
                out=outst[:, 512:1024],
                in0=po[1][:],
                scalar1=0.125,
                scalar2=0.0,
                op0=mybir.AluOpType.mult,
                op1=mybir.AluOpType.max,
            )
            nc.gpsimd.dma_start(
                out_d[s * 128 : (s + 1) * 128, 512:1024], outst[:, 512:1024]
            )

        # warm tiles s0-s2: evac once the chain + their final chunk complete
        for s in (0, 1, 2):
            evac([po_warm[(s, 0)], po_warm[(s, 1)]], s)

        # ---- steady mains + evac/store (s3 half1 first, then s4..s15) ----
        for s in range(3, NS):
            x_cast(s + 1)
            x_cast(s + 2)
            if s == 3:
                po = [po_warm[(3, 0)], None]
                hs = [1]
            else:
                po = [None, None]
                hs = [0, 1]
            for h in hs:
                po[h] = pmm_p.tile(
                    [128, 512], F32, tag="po", bufs=7, name=f"po{s}_{h}"
                )
            for c in range(NT):
                for h in hs:
                    nc.tensor.matmul(
                        po[h][:],
                        lhsT=x_op(s)[:, c * 128 : (c + 1) * 128],
                        rhs=weff_op(c, h),
                        start=(c == 0),
                        stop=(c == NT - 1),
                    )
            evac(po, s)

    nc.compile()
    return nc


# output column unpermute: stored col j=(g,kk,mp) -> true col o=kk*128+g*16+mp
_j = np.arange(D)
_PERM = ((_j >> 4) & 7) * 128 + (_j >> 7) * 16 + (_j & 15)
_INV = np.argsort(_PERM)  # out_full[..., o] = stored[..., _INV[o]]


def _pack_inputs(x_b, adapter_b_b, adapter_a_b, W0):
    """Pure data placement (permutation / replication / zero-padding)."""
    # xcat[dp, s*D + dt*128 + sp] = x[s*128+sp, dt*128+dp]
    xcat = np.ascontiguousarray(
        x_b.reshape(NS, 128, NT, 128)
        .transpose(3, 0, 2, 1)
        .reshape(128, NS * D),
        np.float32,
    )
    # w0cat[op, t*D + ot*128 + dp] = W0[t*128+dp, ot*128+op]
    w0cat = np.ascontiguousarray(
        W0.reshape(NT, 128, NT, 128).transpose(3, 0, 2, 1).reshape(128, NT * D),
        np.float32,
    )
    # aspc[mp*8+rp, g*8+kk] = a[rp, kk*128+g*16+mp]
    mp_i, rp_i, g_i, kk_i = np.meshgrid(
        np.arange(16), np.arange(8), np.arange(8), np.arange(8), indexing="ij"
    )
    aspc = adapter_a_b[rp_i, kk_i * 128 + g_i * 16 + mp_i].reshape(128, 64)
    mmask = (
        np.arange(16)[None, :] == (np.arange(128)[:, None] // 8)
    ).astype(np.float32)
    # bcp[dp, t*8+kk] = b[t*128+dp, kk]
    bcp = adapter_b_b.reshape(NT, 128, R).transpose(1, 0, 2).reshape(128, NT * R)
    cpack = np.ascontiguousarray(
        np.concatenate([aspc, mmask, bcp], axis=1), np.float32
    )
    return {"xcat": xcat, "w0cat": w0cat, "cpack": cpack}


def kernel(x, adapter_b, adapter_a, W0):
    global _compiled
    x = np.asarray(x, np.float32)
    adapter_b = np.asarray(adapter_b, np.float32)
    adapter_a = np.asarray(adapter_a, np.float32)
    W0 = np.asarray(W0, np.float32)
    B = x.shape[0]
    assert B == N_CORES and x.shape == (B, S, D)

    if _compiled is None:
        _compiled = _build_kernel()

    from concourse.bass_utils import run_bass_kernel_spmd

    in_maps = [
        _pack_inputs(x[b], adapter_b[b], adapter_a[b], W0) for b in range(B)
    ]
    res = run_bass_kernel_spmd(_compiled, in_maps, list(range(N_CORES)))
    out = np.stack([res.results[b]["out"] for b in range(B)]).astype(np.float32)
    return out[:, :, _INV]
